# revision 8
# baseline (speedup 1.0000x reference)
"""Two-layer GraphSAGE 'pool' encoder on 8 Trainium2 NeuronCores.

Sharding: edges + source-node features are split across the 8 cores by
source range (layer 1) / by the layer-1 destination owner (layer 2).
Each core projects its source shard (h = relu(x @ Wp + bp)), gathers its
edges' h-rows from local DRAM via indirect DMA in degree-sorted padded
rounds, fuses the edge-weight multiply with the running segment max
(scalar_tensor_tensor mult/max) into an SBUF accumulator over the FULL
destination space, then a ReduceScatter(max) combines the per-core
partial maxima so every core owns a 1/8 destination shard. Output
matmuls run per shard; layer 2 repeats the pattern with the layer-1
output (kept transposed in SBUF). Messages are non-negative (relu * w,
w >= 0), so zero-init accumulators subsume both round padding and the
reference's isolated-destination zeroing.
"""

import sys

for _p in ("/opt/trn_rl_repo",):
    if _p not in sys.path:
        sys.path.insert(0, _p)

import numpy as np

import concourse.bacc as bacc
import concourse.mybir as mybir
import concourse.tile as tile
from concourse.bass import IndirectOffsetOnAxis
from concourse.bass_utils import run_bass_kernel_spmd

NC = 8
N0, N1, N2 = 100000, 20000, 4000
D = 256
SPC0 = N0 // NC           # 12500 layer-1 source rows per core
DPC1 = N1 // NC           # 2500 layer-1 destinations per core (RS shard)
DPC2 = N2 // NC           # 500 layer-2 destinations per core
T1 = -(-N1 // 128)        # 157 accumulator slot-columns, layer 1
T2 = -(-N2 // 128)        # 32 slot-columns, layer 2
WMAX = 8                  # slot-columns per gather call / acc chunk
PHASES1 = [(0, 80), (80, T1)]   # L1 acc split: 80KB/partition per half
PHASES2 = [(0, T2)]
OOB = 2_000_000_000
FP = mybir.dt.float32
I32 = mybir.dt.int32
I16 = mybir.dt.int16


def _core_of_node(s):
    """Owner core of layer-1 destination node s (first 4000 striped 500/core
    so the layer-2 'x_dst' rows are core-local; rest striped 2000/core)."""
    return np.where(s < 4000, s // 500, (s - 4000) // 2000)


def _pos_of_node(s):
    return np.where(s < 4000, s % 500, 500 + (s - 4000) % 2000)


def _build_tables(src_l, dst, w, n_dst, T):
    """Per-core gather/scatter tables for one layer.

    src_l: local source row per edge; dst: destination per edge (natural id).
    Returns (deg_slot [T*128], TAB_idx [T*128, R], TAB_w, node_at_slot)."""
    nslots = T * 128
    deg = np.bincount(dst, minlength=n_dst)
    node_at_slot = np.argsort(-deg, kind="stable")
    slot_of_node = np.empty(n_dst, np.int64)
    slot_of_node[node_at_slot] = np.arange(n_dst)
    deg_slot = np.zeros(nslots, np.int64)
    deg_slot[:n_dst] = deg[node_at_slot]

    slot_e = slot_of_node[dst]
    order_e = np.argsort(slot_e, kind="stable")
    ss = slot_e[order_e]
    new_run = np.r_[True, np.diff(ss) != 0]
    run_starts = np.flatnonzero(new_run)
    run_id = np.cumsum(new_run) - 1
    occ = np.arange(len(ss)) - run_starts[run_id]

    R = int(deg_slot[0]) if len(ss) else 0
    TAB_idx = np.zeros((nslots, max(R, 1)), np.int32)
    TAB_w = np.zeros((nslots, max(R, 1)), np.float32)
    TAB_idx[ss, occ] = src_l[order_e]
    TAB_w[ss, occ] = w[order_e]
    return deg_slot, TAB_idx, TAB_w, node_at_slot


def _shared_widths(deg_slots_per_core):
    """Global active-prefix width (in slot-columns) per round, max over cores."""
    R = max(int(d[0]) for d in deg_slots_per_core)
    return [
        max(-(-int((d > r).sum()) // 128) for d in deg_slots_per_core)
        for r in range(R)
    ]


def _calls_for_range(W_r, tlo, thi):
    """[(r, t0, ncols)] gather calls covering slot-cols [tlo, thi)."""
    calls = []
    for r, wr in enumerate(W_r):
        hi = min(wr, thi)
        t0 = tlo
        while t0 < hi:
            n = min(WMAX - (t0 - tlo) % WMAX, hi - t0)
            calls.append((r, t0, n))
            t0 += n
    return calls


def _wrap16(vals):
    """Logical-order idx list -> [128, n/16] int16 tile (16-wrap, replicated)."""
    n = len(vals)
    w = np.asarray(vals, np.int16).reshape(n // 16, 16).T
    return np.tile(w, (8, 1))


def _pack_gather(TAB_idx, TAB_w, calls, T):
    """Per-core call-order arrays: wrapped idx [128, 8*C] + w [128, C]."""
    R = TAB_idx.shape[1]
    ti = TAB_idx.reshape(T, 128, R)
    tw = TAB_w.reshape(T, 128, R)
    gi, gw = [], []
    for r, t0, n in calls:
        if r < R:
            gi.append(_wrap16(ti[t0 : t0 + n, :, r].reshape(-1)))
            gw.append(tw[t0 : t0 + n, :, r].T)
        else:
            gi.append(np.zeros((128, n * 8), np.int16))
            gw.append(np.zeros((128, n), np.float32))
    return (
        np.ascontiguousarray(np.concatenate(gi, 1)),
        np.ascontiguousarray(np.concatenate(gw, 1)),
    )


def _chunks_for_phases(phases):
    out = []
    for tlo, thi in phases:
        t0 = tlo
        while t0 < thi:
            out.append((t0, min(WMAX, thi - t0)))
            t0 += WMAX
    return out


def _pack_scatter(node_at_slot, gpos, n_dst, T, phases):
    """Wrapped int16 scatter targets in chunk order (-1 trailing phantom)."""
    arr = np.full(T * 128, -1, np.int64)
    arr[:n_dst] = gpos[node_at_slot]
    arr = arr.reshape(T, 128)
    blocks = []
    for t0, w in _chunks_for_phases(phases):
        blocks.append(_wrap16(arr[t0 : t0 + w].reshape(-1)))
    return np.ascontiguousarray(np.concatenate(blocks, 1))


def _prep(inputs):
    x = np.asarray(inputs["x"], np.float32)
    src0 = np.asarray(inputs["src0"], np.int64)
    dst0 = np.asarray(inputs["dst0"], np.int64)
    w0 = np.asarray(inputs["w0"], np.float32)
    src1 = np.asarray(inputs["src1"], np.int64)
    dst1 = np.asarray(inputs["dst1"], np.int64)
    w1 = np.asarray(inputs["w1"], np.float32)

    g1 = _core_of_node(np.arange(N1)) * DPC1 + _pos_of_node(np.arange(N1))

    deg1_all, deg2_all, tabs1, tabs2 = [], [], [], []
    for c in range(NC):
        m = (src0 >= c * SPC0) & (src0 < (c + 1) * SPC0)
        d1, ti1, tw1, slots1 = _build_tables(
            (src0[m] - c * SPC0).astype(np.int32), dst0[m], w0[m], N1, T1
        )
        deg1_all.append(d1)
        tabs1.append((ti1, tw1, slots1))

        mc = _core_of_node(src1) == c
        d2, ti2, tw2, slots2 = _build_tables(
            _pos_of_node(src1[mc]).astype(np.int32), dst1[mc], w1[mc], N2, T2
        )
        deg2_all.append(d2)
        tabs2.append((ti2, tw2, slots2))

    W1 = _shared_widths(deg1_all)
    W2 = _shared_widths(deg2_all)
    calls1 = [c for lo, hi in PHASES1 for c in _calls_for_range(W1, lo, hi)]
    calls2 = [c for lo, hi in PHASES2 for c in _calls_for_range(W2, lo, hi)]

    xT = np.ascontiguousarray(x.T)
    per_core = []
    for c in range(NC):
        ti1, tw1, slots1 = tabs1[c]
        ti2, tw2, slots2 = tabs2[c]
        gi1, gw1 = _pack_gather(ti1, tw1, calls1, T1)
        gi2, gw2 = _pack_gather(ti2, tw2, calls2, T2)
        own = np.r_[
            np.arange(c * 500, (c + 1) * 500),
            np.arange(4000 + c * 2000, 4000 + (c + 1) * 2000),
        ]
        per_core.append(
            {
                "xTp": np.ascontiguousarray(xT[:, c * SPC0 : (c + 1) * SPC0]),
                "xTo": np.ascontiguousarray(xT[:, own]),
                "gidx1": gi1,
                "gw1": gw1,
                "sidx1": _pack_scatter(slots1, g1, N1, T1, PHASES1),
                "gidx2": gi2,
                "gw2": gw2,
                "sidx2": _pack_scatter(slots2, np.arange(N2), N2, T2, PHASES2),
            }
        )

    shared = {
        "Wp1": np.asarray(inputs["Wp1"], np.float32),
        "bp1r": np.asarray(inputs["bp1"], np.float32).reshape(1, D),
        "Ws1": np.asarray(inputs["Ws1"], np.float32),
        "Wn1": np.asarray(inputs["Wn1"], np.float32),
        "b1c": np.asarray(inputs["b1"], np.float32).reshape(D, 1),
        "Wp2": np.asarray(inputs["Wp2"], np.float32),
        "bp2r": np.asarray(inputs["bp2"], np.float32).reshape(1, D),
        "Ws2": np.asarray(inputs["Ws2"], np.float32),
        "Wn2": np.asarray(inputs["Wn2"], np.float32),
        "b2r": np.asarray(inputs["b2"], np.float32).reshape(1, D),
        "ident": np.eye(128, dtype=np.float32),
    }
    in_maps = [{**shared, **pc} for pc in per_core]
    return in_maps, calls1, calls2


def _emit_gather_range(nc, tc, pool, h_dram, gidx, gw, sidx, cc_in, calls,
                       gcol0, sch0, tlo, thi, n_dst, accpool, tagp):
    """One acc range [tlo, thi): memset chunks, dma_gather + fused mul/add
    rounds, per-chunk dma_scatter_add (every chunk scatters)."""
    nch = -(-(thi - tlo) // WMAX)
    accs = []
    for ci in range(nch):
        w = min(WMAX, thi - tlo - ci * WMAX)
        a = accpool.tile([128, w * D], FP, tag=f"{tagp}acc{ci}", name=f"{tagp}acc{ci}")
        nc.vector.memset(a[:], 0.0)
        accs.append((a, w))

    last_call = {}
    my_calls = [(i, c) for i, c in enumerate(calls) if tlo <= c[1] < thi]
    for i, (r, t0, n) in my_calls:
        last_call[(t0 - tlo) // WMAX] = i

    def scatter(ci):
        acc, w = accs[ci]
        t0 = tlo + ci * WMAX
        nvalid = min(w * 128, max(0, n_dst - t0 * 128))
        nc.gpsimd.dma_scatter_add(
            cc_in[:, :],
            acc[:, : w * D].rearrange("p (n e) -> p n e", e=D),
            sidx[:, (sch0 + ci) * WMAX * 8 : (sch0 + ci) * WMAX * 8 + w * 8],
            w * 128, nvalid, D)

    gcol = gcol0
    done = set()
    for i, (r, t0, n) in my_calls:
        ci = (t0 - tlo) // WMAX
        acc, w = accs[ci]
        g = pool.tile([128, WMAX * D], FP, tag="g", name=f"{tagp}g{i}", bufs=4)
        nc.gpsimd.dma_gather(
            g[:, : n * D].rearrange("p (n e) -> p n e", e=D),
            h_dram[:, :],
            gidx[:, gcol * 8 : (gcol + n) * 8],
            n * 128, n * 128, D)
        for j in range(n):
            k = (t0 + j - tlo - ci * WMAX) * D
            nc.vector.scalar_tensor_tensor(
                out=acc[:, k : k + D],
                in0=g[:, j * D : (j + 1) * D],
                scalar=gw[:, gcol + j : gcol + j + 1],
                in1=acc[:, k : k + D],
                op0=mybir.AluOpType.mult,
                op1=mybir.AluOpType.add,
            )
        gcol += n
        if last_call[ci] == i:
            done.add(ci)
            scatter(ci)
    for ci in range(nch):
        if ci not in done:
            scatter(ci)
    return gcol


def _build_program(calls1, calls2, debug=False):
    nc = bacc.Bacc("TRN2", target_bir_lowering=False, debug=False,
                   enable_asserts=True, num_devices=NC)

    xTp_t = nc.dram_tensor("xTp", [D, SPC0], FP, kind="ExternalInput")
    xTo_t = nc.dram_tensor("xTo", [D, DPC1], FP, kind="ExternalInput")
    C1 = sum(n for _, _, n in calls1)
    C2 = sum(n for _, _, n in calls2)
    S1 = sum(w for _, w in _chunks_for_phases(PHASES1)) * 8
    S2 = sum(w for _, w in _chunks_for_phases(PHASES2)) * 8
    gidx1_t = nc.dram_tensor("gidx1", [128, C1 * 8], I16, kind="ExternalInput")
    gw1_t = nc.dram_tensor("gw1", [128, C1], FP, kind="ExternalInput")
    sidx1_t = nc.dram_tensor("sidx1", [128, S1], I16, kind="ExternalInput")
    gidx2_t = nc.dram_tensor("gidx2", [128, C2 * 8], I16, kind="ExternalInput")
    gw2_t = nc.dram_tensor("gw2", [128, C2], FP, kind="ExternalInput")
    sidx2_t = nc.dram_tensor("sidx2", [128, S2], I16, kind="ExternalInput")
    wt = {}
    for name, shape in [
        ("Wp1", [D, D]), ("bp1r", [1, D]), ("Ws1", [D, D]), ("Wn1", [D, D]),
        ("b1c", [D, 1]), ("Wp2", [D, D]), ("bp2r", [1, D]), ("Ws2", [D, D]),
        ("Wn2", [D, D]), ("b2r", [1, D]), ("ident", [128, 128]),
    ]:
        wt[name] = nc.dram_tensor(name, shape, FP, kind="ExternalInput")
    out_t = nc.dram_tensor("out", [DPC2, D], FP, kind="ExternalOutput")

    h_dram = nc.dram_tensor("h_dram", [SPC0, D], FP)
    cc1_in = nc.dram_tensor("cc1_in", [N1, D], FP)
    cc1_out = nc.dram_tensor("cc1_out", [DPC1, D], FP)
    h2_dram = nc.dram_tensor("h2_dram", [DPC1, D], FP)
    cc2_in = nc.dram_tensor("cc2_in", [N2, D], FP)
    cc2_out = nc.dram_tensor("cc2_out", [DPC2, D], FP)
    dbg = {}
    if debug:
        for nm, t in [("h_dram", h_dram), ("cc1_in", cc1_in),
                      ("cc1_out", cc1_out), ("h2_dram", h2_dram),
                      ("cc2_in", cc2_in), ("cc2_out", cc2_out)]:
            dbg[nm] = nc.dram_tensor("dbg_" + nm, list(t.shape), FP,
                                     kind="ExternalOutput")

    Relu = mybir.ActivationFunctionType.Relu
    rg = [list(range(NC))]

    with tile.TileContext(nc) as tc:
        with (
            tc.tile_pool(name="const", bufs=1) as cpool,
            tc.tile_pool(name="work", bufs=3) as pool,
            tc.tile_pool(name="psum", bufs=2, space="PSUM") as psum,
        ):
            # ---- constants in SBUF
            ident = cpool.tile([128, 128], FP, tag="ident")
            nc.sync.dma_start(ident[:], wt["ident"][:, :])
            ones = cpool.tile([1, 128], FP, tag="ones")
            nc.vector.memset(ones[:], 1.0)
            W = {}
            for nm in ("Wp1", "Ws1", "Wn1", "Wp2", "Ws2", "Wn2"):
                W[nm] = cpool.tile([128, 2 * D], FP, tag=nm, name=nm)
                nc.sync.dma_start(W[nm][:, 0:D], wt[nm][0:128, :])
                nc.sync.dma_start(W[nm][:, D : 2 * D], wt[nm][128:256, :])
            brow = {}
            for nm in ("bp1r", "bp2r", "b2r"):
                brow[nm] = cpool.tile([1, D], FP, tag=nm, name=nm)
                nc.sync.dma_start(brow[nm][:], wt[nm][:, :])
            b1c = cpool.tile([128, 2], FP, tag="b1c")
            nc.sync.dma_start(b1c[:, 0:1], wt["b1c"][0:128, :])
            nc.sync.dma_start(b1c[:, 1:2], wt["b1c"][128:256, :])
            gidx1 = cpool.tile([128, C1 * 8], I16, tag="gidx1")
            gw1 = cpool.tile([128, C1], FP, tag="gw1")
            sidx1 = cpool.tile([128, S1], I16, tag="sidx1")
            nc.sync.dma_start(gidx1[:], gidx1_t[:, :])
            nc.sync.dma_start(gw1[:], gw1_t[:, :])
            nc.sync.dma_start(sidx1[:], sidx1_t[:, :])
            gidx2 = cpool.tile([128, C2 * 8], I16, tag="gidx2")
            gw2 = cpool.tile([128, C2], FP, tag="gw2")
            sidx2 = cpool.tile([128, S2], I16, tag="sidx2")
            nc.sync.dma_start(gidx2[:], gidx2_t[:, :])
            nc.sync.dma_start(gw2[:], gw2_t[:, :])
            nc.sync.dma_start(sidx2[:], sidx2_t[:, :])
            h1T = cpool.tile([128, 2 * DPC1], FP, tag="h1T")

            # ---- layer-1 projection: h = relu(x @ Wp1 + bp1), row-major
            with tc.tile_pool(name="xTp", bufs=1) as xpool:
                xTp = xpool.tile([128, 2 * SPC0], FP, tag="xTp")
                nc.sync.dma_start(xTp[:, 0:SPC0], xTp_t[0:128, :])
                nc.sync.dma_start(xTp[:, SPC0 : 2 * SPC0], xTp_t[128:256, :])
                for r0 in range(0, SPC0, 128):
                    nr = min(128, SPC0 - r0)
                    ps = psum.tile([128, D], FP, tag="proj", name=f"ps1_{r0}")
                    nc.tensor.matmul(ps[:nr, :], xTp[:, r0 : r0 + nr],
                                     W["Wp1"][:, 0:D], start=True, stop=False)
                    nc.tensor.matmul(ps[:nr, :],
                                     xTp[:, SPC0 + r0 : SPC0 + r0 + nr],
                                     W["Wp1"][:, D : 2 * D], start=False,
                                     stop=False)
                    nc.tensor.matmul(ps[:nr, :], ones[:, :nr],
                                     brow["bp1r"][:, :], start=False, stop=True)
                    hrow = pool.tile([128, D], FP, tag="hrow", name=f"h1_{r0}")
                    nc.scalar.activation(hrow[:nr, :], ps[:nr, :], Relu)
                    nc.sync.dma_start(h_dram[r0 : r0 + nr, :], hrow[:nr, :])

            # ---- zero collective input buffers (scatter_add accumulates)
            zz = cpool.tile([128, 2048], FP, tag="zz")
            nc.vector.memset(zz[:], 0.0)
            for cc_z, nd in ((cc1_in, N1), (cc2_in, N2)):
                flat = cc_z[:, :].rearrange("(a b) d -> a (b d)", b=8)
                for k0 in range(0, nd // 8, 128):
                    nk = min(128, nd // 8 - k0)
                    nc.sync.dma_start(flat[k0 : k0 + nk, :], zz[:nk, :])

            # ---- layer-1 gather/sum rounds + scatter (two acc ranges)
            gcol = 0
            sch = 0
            for pi, (tlo, thi) in enumerate(PHASES1):
                with tc.tile_pool(name=f"acc1_{pi}", bufs=1) as accpool:
                    gcol = _emit_gather_range(
                        nc, tc, pool, h_dram, gidx1, gw1, sidx1, cc1_in,
                        calls1, gcol, sch, tlo, thi, N1, accpool, f"a1p{pi}")
                    sch += -(-(thi - tlo) // WMAX)

            nc.gpsimd.collective_compute(
                "ReduceScatter", mybir.AluOpType.add, replica_groups=rg,
                ins=[cc1_in[:, :]], outs=[cc1_out[:, :]],
            )

            # ---- layer-1 output: h1T = relu(Ws1.T@xToT + Wn1.T@aggT + b1)
            with tc.tile_pool(name="out1", bufs=1) as opool:
                xTo = opool.tile([128, 2 * DPC1], FP, tag="xTo")
                nc.sync.dma_start(xTo[:, 0:DPC1], xTo_t[0:128, :])
                nc.sync.dma_start(xTo[:, DPC1 : 2 * DPC1], xTo_t[128:256, :])
                aggT = opool.tile([128, 2 * DPC1], FP, tag="aggT")
                for r0 in range(0, DPC1, 128):
                    nr = min(128, DPC1 - r0)
                    ag = pool.tile([128, D], FP, tag="aggrow", name=f"ag_{r0}")
                    nc.sync.dma_start(ag[:nr, :], cc1_out[r0 : r0 + nr, :])
                    for hf in range(2):
                        pt = psum.tile([128, 128], FP, tag="tpose",
                                       name=f"pt_{r0}_{hf}")
                        nc.tensor.transpose(
                            pt[:, :nr], ag[:nr, hf * 128 : (hf + 1) * 128],
                            ident[:nr, :nr])
                        nc.vector.tensor_copy(
                            aggT[:, hf * DPC1 + r0 : hf * DPC1 + r0 + nr],
                            pt[:, :nr])
                for hf in range(2):
                    for n0 in range(0, DPC1, 512):
                        nn = min(512, DPC1 - n0)
                        ps = psum.tile([128, 512], FP, tag="out1",
                                       name=f"po_{hf}_{n0}")
                        for kf in range(2):
                            nc.tensor.matmul(
                                ps[:, :nn],
                                W["Ws1"][:, kf * D + hf * 128 :
                                         kf * D + (hf + 1) * 128],
                                xTo[:, kf * DPC1 + n0 : kf * DPC1 + n0 + nn],
                                start=(kf == 0), stop=False)
                        for kf in range(2):
                            nc.tensor.matmul(
                                ps[:, :nn],
                                W["Wn1"][:, kf * D + hf * 128 :
                                         kf * D + (hf + 1) * 128],
                                aggT[:, kf * DPC1 + n0 : kf * DPC1 + n0 + nn],
                                start=False, stop=(kf == 1))
                        nc.scalar.activation(
                            h1T[:, hf * DPC1 + n0 : hf * DPC1 + n0 + nn],
                            ps[:, :nn], Relu, bias=b1c[:, hf : hf + 1])

            # ---- layer-2 projection: h2 = relu(h1 @ Wp2 + bp2), row-major
            for r0 in range(0, DPC1, 128):
                nr = min(128, DPC1 - r0)
                ps = psum.tile([128, D], FP, tag="proj", name=f"ps2_{r0}")
                for kf in range(2):
                    nc.tensor.matmul(
                        ps[:nr, :],
                        h1T[:, kf * DPC1 + r0 : kf * DPC1 + r0 + nr],
                        W["Wp2"][:, kf * D : (kf + 1) * D],
                        start=(kf == 0), stop=False)
                nc.tensor.matmul(ps[:nr, :], ones[:, :nr], brow["bp2r"][:, :],
                                 start=False, stop=True)
                h2row = pool.tile([128, D], FP, tag="hrow", name=f"h2_{r0}")
                nc.scalar.activation(h2row[:nr, :], ps[:nr, :], Relu)
                nc.sync.dma_start(h2_dram[r0 : r0 + nr, :], h2row[:nr, :])

            # ---- layer-2 gather/sum rounds + scatter
            gcol = 0
            sch = 0
            for pi, (tlo, thi) in enumerate(PHASES2):
                with tc.tile_pool(name=f"acc2_{pi}", bufs=1) as accpool:
                    gcol = _emit_gather_range(
                        nc, tc, pool, h2_dram, gidx2, gw2, sidx2, cc2_in,
                        calls2, gcol, sch, tlo, thi, N2, accpool, f"a2p{pi}")
                    sch += -(-(thi - tlo) // WMAX)

            nc.gpsimd.collective_compute(
                "ReduceScatter", mybir.AluOpType.add, replica_groups=rg,
                ins=[cc2_in[:, :]], outs=[cc2_out[:, :]],
            )

            # ---- layer-2 output: out = relu(h1[:4000]@Ws2 + agg2@Wn2 + b2)
            agg2T = cpool.tile([128, 2 * DPC2], FP, tag="agg2T")
            for r0 in range(0, DPC2, 128):
                nr = min(128, DPC2 - r0)
                ag = pool.tile([128, D], FP, tag="aggrow", name=f"ag2_{r0}")
                nc.sync.dma_start(ag[:nr, :], cc2_out[r0 : r0 + nr, :])
                for hf in range(2):
                    pt = psum.tile([128, 128], FP, tag="tpose",
                                   name=f"pt2_{r0}_{hf}")
                    nc.tensor.transpose(
                        pt[:, :nr], ag[:nr, hf * 128 : (hf + 1) * 128],
                        ident[:nr, :nr])
                    nc.vector.tensor_copy(
                        agg2T[:, hf * DPC2 + r0 : hf * DPC2 + r0 + nr],
                        pt[:, :nr])
            for r0 in range(0, DPC2, 128):
                nr = min(128, DPC2 - r0)
                ps = psum.tile([128, D], FP, tag="proj", name=f"pso_{r0}")
                for kf in range(2):
                    nc.tensor.matmul(
                        ps[:nr, :],
                        h1T[:, kf * DPC1 + r0 : kf * DPC1 + r0 + nr],
                        W["Ws2"][:, kf * D : (kf + 1) * D],
                        start=(kf == 0), stop=False)
                for kf in range(2):
                    nc.tensor.matmul(
                        ps[:nr, :],
                        agg2T[:, kf * DPC2 + r0 : kf * DPC2 + r0 + nr],
                        W["Wn2"][:, kf * D : (kf + 1) * D],
                        start=False, stop=False)
                nc.tensor.matmul(ps[:nr, :], ones[:, :nr], brow["b2r"][:, :],
                                 start=False, stop=True)
                orow = pool.tile([128, D], FP, tag="orow", name=f"o_{r0}")
                nc.scalar.activation(orow[:nr, :], ps[:nr, :], Relu)
                nc.sync.dma_start(out_t[r0 : r0 + nr, :], orow[:nr, :])

            for nm, t in [("h_dram", h_dram), ("cc1_in", cc1_in),
                          ("cc1_out", cc1_out), ("h2_dram", h2_dram),
                          ("cc2_in", cc2_in), ("cc2_out", cc2_out)]:
                if debug:
                    nc.sync.dma_start(dbg[nm][:, :], t[:, :])

    nc.compile()
    return nc


_CACHE = {}


def kernel(**inputs) -> np.ndarray:
    in_maps, calls1, calls2 = _prep(inputs)
    key = (tuple(calls1), tuple(calls2))
    if key not in _CACHE:
        _CACHE[key] = _build_program(calls1, calls2)
    nc = _CACHE[key]
    res = run_bass_kernel_spmd(nc, in_maps, core_ids=list(range(NC)))
    return np.concatenate([res.results[c]["out"] for c in range(NC)], axis=0)


# revision 12
# speedup vs baseline: 1.1618x; 1.1618x over previous
"""Two-layer GraphSAGE 'pool' encoder on 8 Trainium2 NeuronCores.

Sharding: edges + source-node features are split across the 8 cores by
source range (layer 1) / by the layer-1 destination owner (layer 2).
Each core projects its source shard (h = relu(x @ Wp + bp)), gathers its
edges' h-rows from local DRAM via indirect DMA in degree-sorted padded
rounds, fuses the edge-weight multiply with the running segment max
(scalar_tensor_tensor mult/max) into an SBUF accumulator over the FULL
destination space, then a ReduceScatter(max) combines the per-core
partial maxima so every core owns a 1/8 destination shard. Output
matmuls run per shard; layer 2 repeats the pattern with the layer-1
output (kept transposed in SBUF). Messages are non-negative (relu * w,
w >= 0), so zero-init accumulators subsume both round padding and the
reference's isolated-destination zeroing.
"""

import sys

for _p in ("/opt/trn_rl_repo",):
    if _p not in sys.path:
        sys.path.insert(0, _p)

import numpy as np

import concourse.bacc as bacc
import concourse.mybir as mybir
import concourse.tile as tile
from concourse.bass import IndirectOffsetOnAxis
from concourse.bass_utils import run_bass_kernel_spmd

NC = 8
N0, N1, N2 = 100000, 20000, 4000
D = 256
SPC0 = N0 // NC           # 12500 layer-1 source rows per core
DPC1 = N1 // NC           # 2500 layer-1 destinations per core (RS shard)
DPC2 = N2 // NC           # 500 layer-2 destinations per core
T1 = -(-N1 // 128)        # 157 accumulator slot-columns, layer 1
T2 = -(-N2 // 128)        # 32 slot-columns, layer 2
WMAX = 8                  # slot-columns per acc chunk
CALLW = 8                 # columns per dma_gather call (1024-idx HW limit)
PHASES1 = [(0, 80), (80, T1)]   # L1 acc split: 80KB/partition per half
PHASES2 = [(0, T2)]
OOB = 2_000_000_000
FP = mybir.dt.float32
I32 = mybir.dt.int32
I16 = mybir.dt.int16


def _core_of_node(s):
    """Owner core of layer-1 destination node s (first 4000 striped 500/core
    so the layer-2 'x_dst' rows are core-local; rest striped 2000/core)."""
    return np.where(s < 4000, s // 500, (s - 4000) // 2000)


def _pos_of_node(s):
    return np.where(s < 4000, s % 500, 500 + (s - 4000) % 2000)


def _build_tables(src_l, dst, w, n_dst, T):
    """Per-core gather/scatter tables for one layer.

    src_l: local source row per edge; dst: destination per edge (natural id).
    Returns (deg_slot [T*128], TAB_idx [T*128, R], TAB_w, node_at_slot)."""
    nslots = T * 128
    deg = np.bincount(dst, minlength=n_dst)
    node_at_slot = np.argsort(-deg, kind="stable")
    slot_of_node = np.empty(n_dst, np.int64)
    slot_of_node[node_at_slot] = np.arange(n_dst)
    deg_slot = np.zeros(nslots, np.int64)
    deg_slot[:n_dst] = deg[node_at_slot]

    slot_e = slot_of_node[dst]
    order_e = np.argsort(slot_e, kind="stable")
    ss = slot_e[order_e]
    new_run = np.r_[True, np.diff(ss) != 0]
    run_starts = np.flatnonzero(new_run)
    run_id = np.cumsum(new_run) - 1
    occ = np.arange(len(ss)) - run_starts[run_id]

    R = int(deg_slot[0]) if len(ss) else 0
    TAB_idx = np.zeros((nslots, max(R, 1)), np.int32)
    TAB_w = np.zeros((nslots, max(R, 1)), np.float32)
    TAB_idx[ss, occ] = src_l[order_e]
    TAB_w[ss, occ] = w[order_e]
    return deg_slot, TAB_idx, TAB_w, node_at_slot


def _shared_widths(deg_slots_per_core):
    """Global active-prefix width (in slot-columns) per round, max over cores."""
    R = max(int(d[0]) for d in deg_slots_per_core)
    return [
        max(-(-int((d > r).sum()) // 128) for d in deg_slots_per_core)
        for r in range(R)
    ]


def _cols_for_range(W_r, tlo, thi):
    """Flat (r, t) column list covering slot-cols [tlo, thi), r-major."""
    return [(r, t) for r, wr in enumerate(W_r) for t in range(tlo, min(wr, thi))]


def _calls_for_range(W_r, tlo, thi):
    """[[(r, t), ...] <= CALLW cols] gather calls for slot-cols [tlo, thi)."""
    cols = _cols_for_range(W_r, tlo, thi)
    return [tuple(cols[i : i + CALLW]) for i in range(0, len(cols), CALLW)]


def _wrap16(vals):
    """Logical-order idx list -> [128, n/16] int16 tile (16-wrap, replicated)."""
    n = len(vals)
    w = np.asarray(vals, np.int16).reshape(n // 16, 16).T
    return np.tile(w, (8, 1))


def _pack_gather(TAB_idx, TAB_w, calls, T):
    """Per-core call-order arrays: wrapped idx [128, 8*C] + w [128, C]."""
    R = TAB_idx.shape[1]
    ti = TAB_idx.reshape(T, 128, R)
    tw = TAB_w.reshape(T, 128, R)
    gi, gw = [], []
    for call in calls:
        vals = np.zeros((len(call), 128), np.int16)
        wv = np.zeros((128, len(call)), np.float32)
        for j, (r, t) in enumerate(call):
            if r < R:
                vals[j] = ti[t, :, r]
                wv[:, j] = tw[t, :, r]
        gi.append(_wrap16(vals.reshape(-1)))
        gw.append(wv)
    return (
        np.ascontiguousarray(np.concatenate(gi, 1)),
        np.ascontiguousarray(np.concatenate(gw, 1)),
    )


def _chunks_for_phases(phases):
    out = []
    for tlo, thi in phases:
        t0 = tlo
        while t0 < thi:
            out.append((t0, min(WMAX, thi - t0)))
            t0 += WMAX
    return out


def _pack_scatter(node_at_slot, gpos, n_dst, T, phases):
    """Wrapped int16 scatter targets in chunk order (-1 trailing phantom)."""
    arr = np.full(T * 128, -1, np.int64)
    arr[:n_dst] = gpos[node_at_slot]
    arr = arr.reshape(T, 128)
    blocks = []
    for t0, w in _chunks_for_phases(phases):
        blocks.append(_wrap16(arr[t0 : t0 + w].reshape(-1)))
    return np.ascontiguousarray(np.concatenate(blocks, 1))


def _prep(inputs):
    x = np.asarray(inputs["x"], np.float32)
    src0 = np.asarray(inputs["src0"], np.int64)
    dst0 = np.asarray(inputs["dst0"], np.int64)
    w0 = np.asarray(inputs["w0"], np.float32)
    src1 = np.asarray(inputs["src1"], np.int64)
    dst1 = np.asarray(inputs["dst1"], np.int64)
    w1 = np.asarray(inputs["w1"], np.float32)

    g1 = _core_of_node(np.arange(N1)) * DPC1 + _pos_of_node(np.arange(N1))

    deg1_all, deg2_all, tabs1, tabs2 = [], [], [], []
    for c in range(NC):
        m = (src0 >= c * SPC0) & (src0 < (c + 1) * SPC0)
        d1, ti1, tw1, slots1 = _build_tables(
            (src0[m] - c * SPC0).astype(np.int32), dst0[m], w0[m], N1, T1
        )
        deg1_all.append(d1)
        tabs1.append((ti1, tw1, slots1))

        mc = _core_of_node(src1) == c
        d2, ti2, tw2, slots2 = _build_tables(
            _pos_of_node(src1[mc]).astype(np.int32), dst1[mc], w1[mc], N2, T2
        )
        deg2_all.append(d2)
        tabs2.append((ti2, tw2, slots2))

    W1 = _shared_widths(deg1_all)
    W2 = _shared_widths(deg2_all)
    calls1 = [c for lo, hi in PHASES1 for c in _calls_for_range(W1, lo, hi)]
    calls2 = [c for lo, hi in PHASES2 for c in _calls_for_range(W2, lo, hi)]

    xT = np.ascontiguousarray(x.T)
    per_core = []
    for c in range(NC):
        ti1, tw1, slots1 = tabs1[c]
        ti2, tw2, slots2 = tabs2[c]
        gi1, gw1 = _pack_gather(ti1, tw1, calls1, T1)
        gi2, gw2 = _pack_gather(ti2, tw2, calls2, T2)
        own = np.r_[
            np.arange(c * 500, (c + 1) * 500),
            np.arange(4000 + c * 2000, 4000 + (c + 1) * 2000),
        ]
        per_core.append(
            {
                "xTp": np.ascontiguousarray(xT[:, c * SPC0 : (c + 1) * SPC0]),
                "xTo": np.ascontiguousarray(xT[:, own]),
                "gidx1": gi1,
                "gw1": gw1,
                "sidx1": _pack_scatter(slots1, g1, N1, T1, PHASES1),
                "gidx2": gi2,
                "gw2": gw2,
                "sidx2": _pack_scatter(slots2, np.arange(N2), N2, T2, PHASES2),
            }
        )

    shared = {
        "Wp1": np.asarray(inputs["Wp1"], np.float32),
        "bp1r": np.asarray(inputs["bp1"], np.float32).reshape(1, D),
        "Ws1": np.asarray(inputs["Ws1"], np.float32),
        "Wn1": np.asarray(inputs["Wn1"], np.float32),
        "b1c": np.asarray(inputs["b1"], np.float32).reshape(D, 1),
        "Wp2": np.asarray(inputs["Wp2"], np.float32),
        "bp2r": np.asarray(inputs["bp2"], np.float32).reshape(1, D),
        "Ws2": np.asarray(inputs["Ws2"], np.float32),
        "Wn2": np.asarray(inputs["Wn2"], np.float32),
        "b2r": np.asarray(inputs["b2"], np.float32).reshape(1, D),
        "ident": np.eye(128, dtype=np.float32),
    }
    in_maps = [{**shared, **pc} for pc in per_core]
    return in_maps, calls1, calls2


def _emit_gather_range(nc, tc, pool, h_dram, gidx, gw, sidx, cc_in, calls,
                       gcol0, sch0, tlo, thi, n_dst, accpool, tagp):
    """One acc range [tlo, thi): memset chunks, dma_gather (CALLW columns per
    call) + fused mul/add, per-chunk dma_scatter_add."""
    nch = -(-(thi - tlo) // WMAX)
    accs = []
    for ci in range(nch):
        w = min(WMAX, thi - tlo - ci * WMAX)
        a = accpool.tile([128, w * D], FP, tag=f"{tagp}acc{ci}", name=f"{tagp}acc{ci}")
        nc.vector.memset(a[:], 0.0)
        accs.append((a, w))

    my_calls = [(i, c) for i, c in enumerate(calls) if tlo <= c[0][1] < thi]
    last_touch = {}
    for i, call in my_calls:
        for r, t in call:
            last_touch[(t - tlo) // WMAX] = i

    def scatter(ci):
        acc, w = accs[ci]
        t0 = tlo + ci * WMAX
        nvalid = min(w * 128, max(0, n_dst - t0 * 128))
        nc.gpsimd.dma_scatter_add(
            cc_in[:, :],
            acc[:, : w * D].rearrange("p (n e) -> p n e", e=D),
            sidx[:, (sch0 + ci) * WMAX * 8 : (sch0 + ci) * WMAX * 8 + w * 8],
            w * 128, nvalid, D)

    gcol = gcol0
    done = set()
    for i, call in my_calls:
        n = len(call)
        g = pool.tile([128, CALLW * D], FP, tag="g", name=f"{tagp}g{i}", bufs=3)
        nc.gpsimd.dma_gather(
            g[:, : n * D].rearrange("p (n e) -> p n e", e=D),
            h_dram[:, :],
            gidx[:, gcol * 8 : (gcol + n) * 8],
            n * 128, n * 128, D)
        for j, (r, t) in enumerate(call):
            ci = (t - tlo) // WMAX
            acc, w = accs[ci]
            k = (t - tlo - ci * WMAX) * D
            nc.vector.scalar_tensor_tensor(
                out=acc[:, k : k + D],
                in0=g[:, j * D : (j + 1) * D],
                scalar=gw[:, gcol + j : gcol + j + 1],
                in1=acc[:, k : k + D],
                op0=mybir.AluOpType.mult,
                op1=mybir.AluOpType.add,
            )
        gcol += n
        for ci in range(nch):
            if last_touch.get(ci) == i:
                done.add(ci)
                scatter(ci)
    for ci in range(nch):
        if ci not in done:
            scatter(ci)
    return gcol


def _build_program(calls1, calls2, with_bias=True, debug=False):
    nc = bacc.Bacc("TRN2", target_bir_lowering=False, debug=False,
                   enable_asserts=True, num_devices=NC)

    xTp_t = nc.dram_tensor("xTp", [D, SPC0], FP, kind="ExternalInput")
    xTo_t = nc.dram_tensor("xTo", [D, DPC1], FP, kind="ExternalInput")
    C1 = sum(len(c) for c in calls1)
    C2 = sum(len(c) for c in calls2)
    S1 = sum(w for _, w in _chunks_for_phases(PHASES1)) * 8
    S2 = sum(w for _, w in _chunks_for_phases(PHASES2)) * 8
    gidx1_t = nc.dram_tensor("gidx1", [128, C1 * 8], I16, kind="ExternalInput")
    gw1_t = nc.dram_tensor("gw1", [128, C1], FP, kind="ExternalInput")
    sidx1_t = nc.dram_tensor("sidx1", [128, S1], I16, kind="ExternalInput")
    gidx2_t = nc.dram_tensor("gidx2", [128, C2 * 8], I16, kind="ExternalInput")
    gw2_t = nc.dram_tensor("gw2", [128, C2], FP, kind="ExternalInput")
    sidx2_t = nc.dram_tensor("sidx2", [128, S2], I16, kind="ExternalInput")
    wt = {}
    for name, shape in [
        ("Wp1", [D, D]), ("bp1r", [1, D]), ("Ws1", [D, D]), ("Wn1", [D, D]),
        ("b1c", [D, 1]), ("Wp2", [D, D]), ("bp2r", [1, D]), ("Ws2", [D, D]),
        ("Wn2", [D, D]), ("b2r", [1, D]), ("ident", [128, 128]),
    ]:
        wt[name] = nc.dram_tensor(name, shape, FP, kind="ExternalInput")
    out_t = nc.dram_tensor("out", [DPC2, D], FP, kind="ExternalOutput")

    h_dram = nc.dram_tensor("h_dram", [SPC0, D], FP)
    cc1_in = nc.dram_tensor("cc1_in", [N1, D], FP)
    cc1_out = nc.dram_tensor("cc1_out", [DPC1, D], FP)
    h2_dram = nc.dram_tensor("h2_dram", [DPC1, D], FP)
    cc2_in = nc.dram_tensor("cc2_in", [N2, D], FP)
    cc2_out = nc.dram_tensor("cc2_out", [DPC2, D], FP)
    dbg = {}
    if debug:
        for nm, t in [("h_dram", h_dram), ("cc1_in", cc1_in),
                      ("cc1_out", cc1_out), ("h2_dram", h2_dram),
                      ("cc2_in", cc2_in), ("cc2_out", cc2_out)]:
            dbg[nm] = nc.dram_tensor("dbg_" + nm, list(t.shape), FP,
                                     kind="ExternalOutput")

    Relu = mybir.ActivationFunctionType.Relu
    rg = [list(range(NC))]

    with tile.TileContext(nc) as tc:
        with (
            tc.tile_pool(name="const", bufs=1) as cpool,
            tc.tile_pool(name="work", bufs=3) as pool,
            tc.tile_pool(name="psum", bufs=2, space="PSUM") as psum,
        ):
            # ---- constants in SBUF
            ident = cpool.tile([128, 128], FP, tag="ident")
            nc.sync.dma_start(ident[:], wt["ident"][:, :])
            ones = cpool.tile([1, 128], FP, tag="ones")
            nc.vector.memset(ones[:], 1.0)
            W = {}
            for nm in ("Wp1", "Ws1", "Wn1", "Wp2", "Ws2", "Wn2"):
                W[nm] = cpool.tile([128, 2 * D], FP, tag=nm, name=nm)
                nc.sync.dma_start(W[nm][:, 0:D], wt[nm][0:128, :])
                nc.sync.dma_start(W[nm][:, D : 2 * D], wt[nm][128:256, :])
            brow = {}
            for nm in ("bp1r", "bp2r", "b2r"):
                brow[nm] = cpool.tile([1, D], FP, tag=nm, name=nm)
                nc.sync.dma_start(brow[nm][:], wt[nm][:, :])
            b1c = cpool.tile([128, 2], FP, tag="b1c")
            nc.sync.dma_start(b1c[:, 0:1], wt["b1c"][0:128, :])
            nc.sync.dma_start(b1c[:, 1:2], wt["b1c"][128:256, :])
            gidx1 = cpool.tile([128, C1 * 8], I16, tag="gidx1")
            gw1 = cpool.tile([128, C1], FP, tag="gw1")
            sidx1 = cpool.tile([128, S1], I16, tag="sidx1")
            nc.sync.dma_start(gidx1[:], gidx1_t[:, :])
            nc.sync.dma_start(gw1[:], gw1_t[:, :])
            nc.sync.dma_start(sidx1[:], sidx1_t[:, :])
            gidx2 = cpool.tile([128, C2 * 8], I16, tag="gidx2")
            gw2 = cpool.tile([128, C2], FP, tag="gw2")
            sidx2 = cpool.tile([128, S2], I16, tag="sidx2")
            nc.sync.dma_start(gidx2[:], gidx2_t[:, :])
            nc.sync.dma_start(gw2[:], gw2_t[:, :])
            nc.sync.dma_start(sidx2[:], sidx2_t[:, :])
            h1T = cpool.tile([128, 2 * DPC1], FP, tag="h1T")

            # ---- layer-1 projection: h = relu(x @ Wp1 + bp1), row-major
            with tc.tile_pool(name="xTp", bufs=1) as xpool:
                xTp = xpool.tile([128, 2 * SPC0], FP, tag="xTp")
                nc.sync.dma_start(xTp[:, 0:SPC0], xTp_t[0:128, :])
                nc.sync.dma_start(xTp[:, SPC0 : 2 * SPC0], xTp_t[128:256, :])
                for r0 in range(0, SPC0, 128):
                    nr = min(128, SPC0 - r0)
                    ps = psum.tile([128, D], FP, tag="proj", name=f"ps1_{r0}")
                    nc.tensor.matmul(ps[:nr, :], xTp[:, r0 : r0 + nr],
                                     W["Wp1"][:, 0:D], start=True, stop=False)
                    nc.tensor.matmul(ps[:nr, :],
                                     xTp[:, SPC0 + r0 : SPC0 + r0 + nr],
                                     W["Wp1"][:, D : 2 * D], start=False,
                                     stop=not with_bias)
                    if with_bias:
                        nc.tensor.matmul(ps[:nr, :], ones[:, :nr],
                                         brow["bp1r"][:, :], start=False,
                                         stop=True)
                    hrow = pool.tile([128, D], FP, tag="hrow", name=f"h1_{r0}")
                    nc.scalar.activation(hrow[:nr, :], ps[:nr, :], Relu)
                    nc.sync.dma_start(h_dram[r0 : r0 + nr, :], hrow[:nr, :])

            # ---- zero collective input buffers (scatter_add accumulates)
            zz = cpool.tile([128, 512], FP, tag="zz")
            nc.vector.memset(zz[:], 0.0)
            for cc_z, nd in ((cc1_in, N1), (cc2_in, N2)):
                flat = cc_z[:, :].rearrange("(a b) d -> a (b d)", b=2)
                for k0 in range(0, nd // 2, 128):
                    nk = min(128, nd // 2 - k0)
                    nc.sync.dma_start(flat[k0 : k0 + nk, :], zz[:nk, :])

            # ---- layer-1 gather/sum rounds + scatter (two acc ranges)
            gcol = 0
            sch = 0
            for pi, (tlo, thi) in enumerate(PHASES1):
                with tc.tile_pool(name=f"acc1_{pi}", bufs=1) as accpool:
                    gcol = _emit_gather_range(
                        nc, tc, pool, h_dram, gidx1, gw1, sidx1, cc1_in,
                        calls1, gcol, sch, tlo, thi, N1, accpool, f"a1p{pi}")
                    sch += -(-(thi - tlo) // WMAX)

            nc.gpsimd.collective_compute(
                "ReduceScatter", mybir.AluOpType.add, replica_groups=rg,
                ins=[cc1_in[:, :]], outs=[cc1_out[:, :]],
            )

            # ---- layer-1 output: h1T = relu(Ws1.T@xToT + Wn1.T@aggT + b1)
            with tc.tile_pool(name="out1", bufs=1) as opool:
                xTo = opool.tile([128, 2 * DPC1], FP, tag="xTo")
                nc.sync.dma_start(xTo[:, 0:DPC1], xTo_t[0:128, :])
                nc.sync.dma_start(xTo[:, DPC1 : 2 * DPC1], xTo_t[128:256, :])
                aggT = opool.tile([128, 2 * DPC1], FP, tag="aggT")
                for r0 in range(0, DPC1, 128):
                    nr = min(128, DPC1 - r0)
                    ag = pool.tile([128, D], FP, tag="aggrow", name=f"ag_{r0}")
                    nc.sync.dma_start(ag[:nr, :], cc1_out[r0 : r0 + nr, :])
                    for hf in range(2):
                        pt = psum.tile([128, 128], FP, tag="tpose",
                                       name=f"pt_{r0}_{hf}")
                        nc.tensor.transpose(
                            pt[:, :nr], ag[:nr, hf * 128 : (hf + 1) * 128],
                            ident[:nr, :nr])
                        nc.vector.tensor_copy(
                            aggT[:, hf * DPC1 + r0 : hf * DPC1 + r0 + nr],
                            pt[:, :nr])
                for hf in range(2):
                    for n0 in range(0, DPC1, 512):
                        nn = min(512, DPC1 - n0)
                        ps = psum.tile([128, 512], FP, tag="out1",
                                       name=f"po_{hf}_{n0}")
                        for kf in range(2):
                            nc.tensor.matmul(
                                ps[:, :nn],
                                W["Ws1"][:, kf * D + hf * 128 :
                                         kf * D + (hf + 1) * 128],
                                xTo[:, kf * DPC1 + n0 : kf * DPC1 + n0 + nn],
                                start=(kf == 0), stop=False)
                        for kf in range(2):
                            nc.tensor.matmul(
                                ps[:, :nn],
                                W["Wn1"][:, kf * D + hf * 128 :
                                         kf * D + (hf + 1) * 128],
                                aggT[:, kf * DPC1 + n0 : kf * DPC1 + n0 + nn],
                                start=False, stop=(kf == 1))
                        nc.scalar.activation(
                            h1T[:, hf * DPC1 + n0 : hf * DPC1 + n0 + nn],
                            ps[:, :nn], Relu, bias=b1c[:, hf : hf + 1])

            # ---- layer-2 projection: h2 = relu(h1 @ Wp2 + bp2), row-major
            for r0 in range(0, DPC1, 128):
                nr = min(128, DPC1 - r0)
                ps = psum.tile([128, D], FP, tag="proj", name=f"ps2_{r0}")
                for kf in range(2):
                    nc.tensor.matmul(
                        ps[:nr, :],
                        h1T[:, kf * DPC1 + r0 : kf * DPC1 + r0 + nr],
                        W["Wp2"][:, kf * D : (kf + 1) * D],
                        start=(kf == 0), stop=False)
                if with_bias:
                    nc.tensor.matmul(ps[:nr, :], ones[:, :nr],
                                     brow["bp2r"][:, :], start=False, stop=True)
                h2row = pool.tile([128, D], FP, tag="hrow", name=f"h2_{r0}")
                nc.scalar.activation(h2row[:nr, :], ps[:nr, :], Relu)
                nc.sync.dma_start(h2_dram[r0 : r0 + nr, :], h2row[:nr, :])

            # ---- layer-2 gather/sum rounds + scatter
            gcol = 0
            sch = 0
            for pi, (tlo, thi) in enumerate(PHASES2):
                with tc.tile_pool(name=f"acc2_{pi}", bufs=1) as accpool:
                    gcol = _emit_gather_range(
                        nc, tc, pool, h2_dram, gidx2, gw2, sidx2, cc2_in,
                        calls2, gcol, sch, tlo, thi, N2, accpool, f"a2p{pi}")
                    sch += -(-(thi - tlo) // WMAX)

            nc.gpsimd.collective_compute(
                "ReduceScatter", mybir.AluOpType.add, replica_groups=rg,
                ins=[cc2_in[:, :]], outs=[cc2_out[:, :]],
            )

            # ---- layer-2 output: out = relu(h1[:4000]@Ws2 + agg2@Wn2 + b2)
            agg2T = cpool.tile([128, 2 * DPC2], FP, tag="agg2T")
            for r0 in range(0, DPC2, 128):
                nr = min(128, DPC2 - r0)
                ag = pool.tile([128, D], FP, tag="aggrow", name=f"ag2_{r0}")
                nc.sync.dma_start(ag[:nr, :], cc2_out[r0 : r0 + nr, :])
                for hf in range(2):
                    pt = psum.tile([128, 128], FP, tag="tpose",
                                   name=f"pt2_{r0}_{hf}")
                    nc.tensor.transpose(
                        pt[:, :nr], ag[:nr, hf * 128 : (hf + 1) * 128],
                        ident[:nr, :nr])
                    nc.vector.tensor_copy(
                        agg2T[:, hf * DPC2 + r0 : hf * DPC2 + r0 + nr],
                        pt[:, :nr])
            for r0 in range(0, DPC2, 128):
                nr = min(128, DPC2 - r0)
                ps = psum.tile([128, D], FP, tag="proj", name=f"pso_{r0}")
                for kf in range(2):
                    nc.tensor.matmul(
                        ps[:nr, :],
                        h1T[:, kf * DPC1 + r0 : kf * DPC1 + r0 + nr],
                        W["Ws2"][:, kf * D : (kf + 1) * D],
                        start=(kf == 0), stop=False)
                for kf in range(2):
                    nc.tensor.matmul(
                        ps[:nr, :],
                        agg2T[:, kf * DPC2 + r0 : kf * DPC2 + r0 + nr],
                        W["Wn2"][:, kf * D : (kf + 1) * D],
                        start=False, stop=(kf == 1 and not with_bias))
                if with_bias:
                    nc.tensor.matmul(ps[:nr, :], ones[:, :nr], brow["b2r"][:, :],
                                     start=False, stop=True)
                orow = pool.tile([128, D], FP, tag="orow", name=f"o_{r0}")
                nc.scalar.activation(orow[:nr, :], ps[:nr, :], Relu)
                nc.sync.dma_start(out_t[r0 : r0 + nr, :], orow[:nr, :])

            for nm, t in [("h_dram", h_dram), ("cc1_in", cc1_in),
                          ("cc1_out", cc1_out), ("h2_dram", h2_dram),
                          ("cc2_in", cc2_in), ("cc2_out", cc2_out)]:
                if debug:
                    nc.sync.dma_start(dbg[nm][:, :], t[:, :])

    nc.compile()
    return nc


_CACHE = {}


def kernel(**inputs) -> np.ndarray:
    in_maps, calls1, calls2 = _prep(inputs)
    with_bias = any(
        np.any(np.asarray(inputs[b])) for b in ("bp1", "b1", "bp2", "b2"))
    key = (tuple(calls1), tuple(calls2), with_bias)
    if key not in _CACHE:
        _CACHE[key] = _build_program(calls1, calls2, with_bias=with_bias)
    nc = _CACHE[key]
    res = run_bass_kernel_spmd(nc, in_maps, core_ids=list(range(NC)))
    return np.concatenate([res.results[c]["out"] for c in range(NC)], axis=0)


# revision 13
# speedup vs baseline: 1.2200x; 1.0501x over previous
"""Two-layer GraphSAGE 'pool' encoder on 8 Trainium2 NeuronCores.

Sharding: edges + source-node features are split across the 8 cores by
source range (layer 1) / by the layer-1 destination owner (layer 2).
Each core projects its source shard (h = relu(x @ Wp + bp)), gathers its
edges' h-rows from local DRAM via indirect DMA in degree-sorted padded
rounds, fuses the edge-weight multiply with the running segment max
(scalar_tensor_tensor mult/max) into an SBUF accumulator over the FULL
destination space, then a ReduceScatter(max) combines the per-core
partial maxima so every core owns a 1/8 destination shard. Output
matmuls run per shard; layer 2 repeats the pattern with the layer-1
output (kept transposed in SBUF). Messages are non-negative (relu * w,
w >= 0), so zero-init accumulators subsume both round padding and the
reference's isolated-destination zeroing.
"""

import sys

for _p in ("/opt/trn_rl_repo",):
    if _p not in sys.path:
        sys.path.insert(0, _p)

import numpy as np
import ml_dtypes

import concourse.bacc as bacc
import concourse.mybir as mybir
import concourse.tile as tile
from concourse.bass import IndirectOffsetOnAxis
from concourse.bass_utils import run_bass_kernel_spmd

NC = 8
N0, N1, N2 = 100000, 20000, 4000
D = 256
SPC0 = N0 // NC           # 12500 layer-1 source rows per core
DPC1 = N1 // NC           # 2500 layer-1 destinations per core (RS shard)
DPC2 = N2 // NC           # 500 layer-2 destinations per core
T1 = -(-N1 // 128)        # 157 accumulator slot-columns, layer 1
T2 = -(-N2 // 128)        # 32 slot-columns, layer 2
WMAX = 8                  # slot-columns per acc chunk
CALLW = 8                 # columns per dma_gather call (1024-idx HW limit)
PHASES1 = [(0, 80), (80, T1)]   # L1 acc split: 80KB/partition per half
PHASES2 = [(0, T2)]
OOB = 2_000_000_000
FP = mybir.dt.float32
BF = mybir.dt.bfloat16
NPBF = ml_dtypes.bfloat16
I32 = mybir.dt.int32
I16 = mybir.dt.int16


def _core_of_node(s):
    """Owner core of layer-1 destination node s (first 4000 striped 500/core
    so the layer-2 'x_dst' rows are core-local; rest striped 2000/core)."""
    return np.where(s < 4000, s // 500, (s - 4000) // 2000)


def _pos_of_node(s):
    return np.where(s < 4000, s % 500, 500 + (s - 4000) % 2000)


def _build_tables(src_l, dst, w, n_dst, T):
    """Per-core gather/scatter tables for one layer.

    src_l: local source row per edge; dst: destination per edge (natural id).
    Returns (deg_slot [T*128], TAB_idx [T*128, R], TAB_w, node_at_slot)."""
    nslots = T * 128
    deg = np.bincount(dst, minlength=n_dst)
    node_at_slot = np.argsort(-deg, kind="stable")
    slot_of_node = np.empty(n_dst, np.int64)
    slot_of_node[node_at_slot] = np.arange(n_dst)
    deg_slot = np.zeros(nslots, np.int64)
    deg_slot[:n_dst] = deg[node_at_slot]

    slot_e = slot_of_node[dst]
    order_e = np.argsort(slot_e, kind="stable")
    ss = slot_e[order_e]
    new_run = np.r_[True, np.diff(ss) != 0]
    run_starts = np.flatnonzero(new_run)
    run_id = np.cumsum(new_run) - 1
    occ = np.arange(len(ss)) - run_starts[run_id]

    R = int(deg_slot[0]) if len(ss) else 0
    TAB_idx = np.zeros((nslots, max(R, 1)), np.int32)
    TAB_w = np.zeros((nslots, max(R, 1)), np.float32)
    TAB_idx[ss, occ] = src_l[order_e]
    TAB_w[ss, occ] = w[order_e]
    return deg_slot, TAB_idx, TAB_w, node_at_slot


def _shared_widths(deg_slots_per_core):
    """Global active-prefix width (in slot-columns) per round, max over cores."""
    R = max(int(d[0]) for d in deg_slots_per_core)
    return [
        max(-(-int((d > r).sum()) // 128) for d in deg_slots_per_core)
        for r in range(R)
    ]


def _cols_for_range(W_r, tlo, thi):
    """Flat (r, t) column list covering slot-cols [tlo, thi), r-major."""
    return [(r, t) for r, wr in enumerate(W_r) for t in range(tlo, min(wr, thi))]


def _calls_for_range(W_r, tlo, thi):
    """[[(r, t), ...] <= CALLW cols] gather calls for slot-cols [tlo, thi)."""
    cols = _cols_for_range(W_r, tlo, thi)
    return [tuple(cols[i : i + CALLW]) for i in range(0, len(cols), CALLW)]


def _wrap16(vals):
    """Logical-order idx list -> [128, n/16] int16 tile (16-wrap, replicated)."""
    n = len(vals)
    w = np.asarray(vals, np.int16).reshape(n // 16, 16).T
    return np.tile(w, (8, 1))


def _pack_gather(TAB_idx, TAB_w, calls, T):
    """Per-core call-order arrays: wrapped idx [128, 8*C] + w [128, C]."""
    R = TAB_idx.shape[1]
    ti = TAB_idx.reshape(T, 128, R)
    tw = TAB_w.reshape(T, 128, R)
    gi, gw = [], []
    for call in calls:
        vals = np.zeros((len(call), 128), np.int16)
        wv = np.zeros((128, len(call)), np.float32)
        for j, (r, t) in enumerate(call):
            if r < R:
                vals[j] = ti[t, :, r]
                wv[:, j] = tw[t, :, r]
        gi.append(_wrap16(vals.reshape(-1)))
        gw.append(wv)
    return (
        np.ascontiguousarray(np.concatenate(gi, 1)),
        np.ascontiguousarray(np.concatenate(gw, 1)),
    )


def _chunks_for_phases(phases):
    out = []
    for tlo, thi in phases:
        t0 = tlo
        while t0 < thi:
            out.append((t0, min(WMAX, thi - t0)))
            t0 += WMAX
    return out


def _pack_scatter(node_at_slot, gpos, n_dst, T, phases):
    """Wrapped int16 scatter targets in chunk order (-1 trailing phantom)."""
    arr = np.full(T * 128, -1, np.int64)
    arr[:n_dst] = gpos[node_at_slot]
    arr = arr.reshape(T, 128)
    blocks = []
    for t0, w in _chunks_for_phases(phases):
        blocks.append(_wrap16(arr[t0 : t0 + w].reshape(-1)))
    return np.ascontiguousarray(np.concatenate(blocks, 1))


def _prep(inputs):
    x = np.asarray(inputs["x"], np.float32)
    src0 = np.asarray(inputs["src0"], np.int64)
    dst0 = np.asarray(inputs["dst0"], np.int64)
    w0 = np.asarray(inputs["w0"], np.float32)
    src1 = np.asarray(inputs["src1"], np.int64)
    dst1 = np.asarray(inputs["dst1"], np.int64)
    w1 = np.asarray(inputs["w1"], np.float32)

    g1 = _core_of_node(np.arange(N1)) * DPC1 + _pos_of_node(np.arange(N1))

    deg1_all, deg2_all, tabs1, tabs2 = [], [], [], []
    for c in range(NC):
        m = (src0 >= c * SPC0) & (src0 < (c + 1) * SPC0)
        d1, ti1, tw1, slots1 = _build_tables(
            (src0[m] - c * SPC0).astype(np.int32), dst0[m], w0[m], N1, T1
        )
        deg1_all.append(d1)
        tabs1.append((ti1, tw1, slots1))

        mc = _core_of_node(src1) == c
        d2, ti2, tw2, slots2 = _build_tables(
            _pos_of_node(src1[mc]).astype(np.int32), dst1[mc], w1[mc], N2, T2
        )
        deg2_all.append(d2)
        tabs2.append((ti2, tw2, slots2))

    W1 = _shared_widths(deg1_all)
    W2 = _shared_widths(deg2_all)
    calls1 = [c for lo, hi in PHASES1 for c in _calls_for_range(W1, lo, hi)]
    calls2 = [c for lo, hi in PHASES2 for c in _calls_for_range(W2, lo, hi)]

    xT = np.ascontiguousarray(x.T)
    per_core = []
    for c in range(NC):
        ti1, tw1, slots1 = tabs1[c]
        ti2, tw2, slots2 = tabs2[c]
        gi1, gw1 = _pack_gather(ti1, tw1, calls1, T1)
        gi2, gw2 = _pack_gather(ti2, tw2, calls2, T2)
        own = np.r_[
            np.arange(c * 500, (c + 1) * 500),
            np.arange(4000 + c * 2000, 4000 + (c + 1) * 2000),
        ]
        per_core.append(
            {
                "xTp": np.ascontiguousarray(xT[:, c * SPC0 : (c + 1) * SPC0]).astype(NPBF),
                "xTo": np.ascontiguousarray(xT[:, own]).astype(NPBF),
                "gidx1": gi1,
                "gw1": gw1,
                "sidx1": _pack_scatter(slots1, g1, N1, T1, PHASES1),
                "gidx2": gi2,
                "gw2": gw2,
                "sidx2": _pack_scatter(slots2, np.arange(N2), N2, T2, PHASES2),
            }
        )

    shared = {
        "Wp1": np.asarray(inputs["Wp1"], np.float32).astype(NPBF),
        "bp1r": np.asarray(inputs["bp1"], np.float32).astype(NPBF).reshape(1, D),
        "Ws1": np.asarray(inputs["Ws1"], np.float32).astype(NPBF),
        "Wn1": np.asarray(inputs["Wn1"], np.float32).astype(NPBF),
        "b1c": np.asarray(inputs["b1"], np.float32).reshape(D, 1),
        "Wp2": np.asarray(inputs["Wp2"], np.float32).astype(NPBF),
        "bp2r": np.asarray(inputs["bp2"], np.float32).astype(NPBF).reshape(1, D),
        "Ws2": np.asarray(inputs["Ws2"], np.float32).astype(NPBF),
        "Wn2": np.asarray(inputs["Wn2"], np.float32).astype(NPBF),
        "b2r": np.asarray(inputs["b2"], np.float32).astype(NPBF).reshape(1, D),
        "ident": np.eye(128, dtype=np.float32),
    }
    in_maps = [{**shared, **pc} for pc in per_core]
    return in_maps, calls1, calls2


def _emit_gather_range(nc, tc, pool, h_dram, gidx, gw, sidx, cc_in, calls,
                       gcol0, sch0, tlo, thi, n_dst, accpool, tagp):
    """One acc range [tlo, thi): memset chunks, dma_gather (CALLW columns per
    call) + fused mul/add, per-chunk dma_scatter_add."""
    nch = -(-(thi - tlo) // WMAX)
    accs = []
    for ci in range(nch):
        w = min(WMAX, thi - tlo - ci * WMAX)
        a = accpool.tile([128, w * D], FP, tag=f"{tagp}acc{ci}", name=f"{tagp}acc{ci}")
        nc.vector.memset(a[:], 0.0)
        accs.append((a, w))

    my_calls = [(i, c) for i, c in enumerate(calls) if tlo <= c[0][1] < thi]
    last_touch = {}
    for i, call in my_calls:
        for r, t in call:
            last_touch[(t - tlo) // WMAX] = i

    def scatter(ci):
        acc, w = accs[ci]
        t0 = tlo + ci * WMAX
        nvalid = min(w * 128, max(0, n_dst - t0 * 128))
        nc.gpsimd.dma_scatter_add(
            cc_in[:, :],
            acc[:, : w * D].rearrange("p (n e) -> p n e", e=D),
            sidx[:, (sch0 + ci) * WMAX * 8 : (sch0 + ci) * WMAX * 8 + w * 8],
            w * 128, nvalid, D)

    gcol = gcol0
    done = set()
    for i, call in my_calls:
        n = len(call)
        g = pool.tile([128, CALLW * D], BF, tag="g", name=f"{tagp}g{i}", bufs=3)
        nc.gpsimd.dma_gather(
            g[:, : n * D].rearrange("p (n e) -> p n e", e=D),
            h_dram[:, :],
            gidx[:, gcol * 8 : (gcol + n) * 8],
            n * 128, n * 128, D)
        for j, (r, t) in enumerate(call):
            ci = (t - tlo) // WMAX
            acc, w = accs[ci]
            k = (t - tlo - ci * WMAX) * D
            nc.vector.scalar_tensor_tensor(
                out=acc[:, k : k + D],
                in0=g[:, j * D : (j + 1) * D],
                scalar=gw[:, gcol + j : gcol + j + 1],
                in1=acc[:, k : k + D],
                op0=mybir.AluOpType.mult,
                op1=mybir.AluOpType.add,
            )
        gcol += n
        for ci in range(nch):
            if last_touch.get(ci) == i:
                done.add(ci)
                scatter(ci)
    for ci in range(nch):
        if ci not in done:
            scatter(ci)
    return gcol


def _build_program(calls1, calls2, with_bias=True, debug=False):
    nc = bacc.Bacc("TRN2", target_bir_lowering=False, debug=False,
                   enable_asserts=True, num_devices=NC)

    xTp_t = nc.dram_tensor("xTp", [D, SPC0], BF, kind="ExternalInput")
    xTo_t = nc.dram_tensor("xTo", [D, DPC1], BF, kind="ExternalInput")
    C1 = sum(len(c) for c in calls1)
    C2 = sum(len(c) for c in calls2)
    S1 = sum(w for _, w in _chunks_for_phases(PHASES1)) * 8
    S2 = sum(w for _, w in _chunks_for_phases(PHASES2)) * 8
    gidx1_t = nc.dram_tensor("gidx1", [128, C1 * 8], I16, kind="ExternalInput")
    gw1_t = nc.dram_tensor("gw1", [128, C1], FP, kind="ExternalInput")
    sidx1_t = nc.dram_tensor("sidx1", [128, S1], I16, kind="ExternalInput")
    gidx2_t = nc.dram_tensor("gidx2", [128, C2 * 8], I16, kind="ExternalInput")
    gw2_t = nc.dram_tensor("gw2", [128, C2], FP, kind="ExternalInput")
    sidx2_t = nc.dram_tensor("sidx2", [128, S2], I16, kind="ExternalInput")
    wt = {}
    for name, shape in [
        ("Wp1", [D, D]), ("bp1r", [1, D]), ("Ws1", [D, D]), ("Wn1", [D, D]),
        ("Wp2", [D, D]), ("bp2r", [1, D]), ("Ws2", [D, D]),
        ("Wn2", [D, D]), ("b2r", [1, D]),
    ]:
        wt[name] = nc.dram_tensor(name, shape, BF, kind="ExternalInput")
    wt["b1c"] = nc.dram_tensor("b1c", [D, 1], FP, kind="ExternalInput")
    wt["ident"] = nc.dram_tensor("ident", [128, 128], FP, kind="ExternalInput")
    out_t = nc.dram_tensor("out", [DPC2, D], FP, kind="ExternalOutput")

    h_dram = nc.dram_tensor("h_dram", [SPC0, D], BF)
    cc1_in = nc.dram_tensor("cc1_in", [N1, D], FP)
    cc1_out = nc.dram_tensor("cc1_out", [DPC1, D], FP)
    h2_dram = nc.dram_tensor("h2_dram", [DPC1, D], BF)
    cc2_in = nc.dram_tensor("cc2_in", [N2, D], FP)
    cc2_out = nc.dram_tensor("cc2_out", [DPC2, D], FP)
    dbg = {}
    if debug:
        for nm, t in [("h_dram", h_dram), ("cc1_in", cc1_in),
                      ("cc1_out", cc1_out), ("h2_dram", h2_dram),
                      ("cc2_in", cc2_in), ("cc2_out", cc2_out)]:
            dbg[nm] = nc.dram_tensor("dbg_" + nm, list(t.shape), FP,
                                     kind="ExternalOutput")

    Relu = mybir.ActivationFunctionType.Relu
    rg = [list(range(NC))]

    with tile.TileContext(nc) as tc:
        with (
            tc.tile_pool(name="const", bufs=1) as cpool,
            tc.tile_pool(name="work", bufs=3) as pool,
            tc.tile_pool(name="psum", bufs=2, space="PSUM") as psum,
        ):
            # ---- constants in SBUF
            ident = cpool.tile([128, 128], FP, tag="ident")
            nc.sync.dma_start(ident[:], wt["ident"][:, :])
            ones = cpool.tile([1, 128], BF, tag="ones")
            nc.vector.memset(ones[:], 1.0)
            W = {}
            for nm in ("Wp1", "Ws1", "Wn1", "Wp2", "Ws2", "Wn2"):
                W[nm] = cpool.tile([128, 2 * D], BF, tag=nm, name=nm)
                nc.sync.dma_start(W[nm][:, 0:D], wt[nm][0:128, :])
                nc.sync.dma_start(W[nm][:, D : 2 * D], wt[nm][128:256, :])
            brow = {}
            for nm in ("bp1r", "bp2r", "b2r"):
                brow[nm] = cpool.tile([1, D], BF, tag=nm, name=nm)
                nc.sync.dma_start(brow[nm][:], wt[nm][:, :])
            b1c = cpool.tile([128, 2], FP, tag="b1c")
            nc.sync.dma_start(b1c[:, 0:1], wt["b1c"][0:128, :])
            nc.sync.dma_start(b1c[:, 1:2], wt["b1c"][128:256, :])
            gidx1 = cpool.tile([128, C1 * 8], I16, tag="gidx1")
            gw1 = cpool.tile([128, C1], FP, tag="gw1")
            sidx1 = cpool.tile([128, S1], I16, tag="sidx1")
            nc.sync.dma_start(gidx1[:], gidx1_t[:, :])
            nc.sync.dma_start(gw1[:], gw1_t[:, :])
            nc.sync.dma_start(sidx1[:], sidx1_t[:, :])
            gidx2 = cpool.tile([128, C2 * 8], I16, tag="gidx2")
            gw2 = cpool.tile([128, C2], FP, tag="gw2")
            sidx2 = cpool.tile([128, S2], I16, tag="sidx2")
            nc.sync.dma_start(gidx2[:], gidx2_t[:, :])
            nc.sync.dma_start(gw2[:], gw2_t[:, :])
            nc.sync.dma_start(sidx2[:], sidx2_t[:, :])
            h1T = cpool.tile([128, 2 * DPC1], BF, tag="h1T")

            # ---- layer-1 projection: h = relu(x @ Wp1 + bp1), row-major
            with tc.tile_pool(name="xTp", bufs=1) as xpool:
                xTp = xpool.tile([128, 2 * SPC0], BF, tag="xTp")
                nc.sync.dma_start(xTp[:, 0:SPC0], xTp_t[0:128, :])
                nc.sync.dma_start(xTp[:, SPC0 : 2 * SPC0], xTp_t[128:256, :])
                for r0 in range(0, SPC0, 128):
                    nr = min(128, SPC0 - r0)
                    ps = psum.tile([128, D], FP, tag="proj", name=f"ps1_{r0}")
                    nc.tensor.matmul(ps[:nr, :], xTp[:, r0 : r0 + nr],
                                     W["Wp1"][:, 0:D], start=True, stop=False)
                    nc.tensor.matmul(ps[:nr, :],
                                     xTp[:, SPC0 + r0 : SPC0 + r0 + nr],
                                     W["Wp1"][:, D : 2 * D], start=False,
                                     stop=not with_bias)
                    if with_bias:
                        nc.tensor.matmul(ps[:nr, :], ones[:, :nr],
                                         brow["bp1r"][:, :], start=False,
                                         stop=True)
                    hrow = pool.tile([128, D], BF, tag="hrow", name=f"h1_{r0}")
                    nc.scalar.activation(hrow[:nr, :], ps[:nr, :], Relu)
                    nc.sync.dma_start(h_dram[r0 : r0 + nr, :], hrow[:nr, :])

            # ---- zero collective input buffers (scatter_add accumulates)
            zz = cpool.tile([128, 512], FP, tag="zz")
            nc.vector.memset(zz[:], 0.0)
            for cc_z, nd in ((cc1_in, N1), (cc2_in, N2)):
                flat = cc_z[:, :].rearrange("(a b) d -> a (b d)", b=2)
                for k0 in range(0, nd // 2, 128):
                    nk = min(128, nd // 2 - k0)
                    nc.sync.dma_start(flat[k0 : k0 + nk, :], zz[:nk, :])

            # ---- layer-1 gather/sum rounds + scatter (two acc ranges)
            gcol = 0
            sch = 0
            for pi, (tlo, thi) in enumerate(PHASES1):
                with tc.tile_pool(name=f"acc1_{pi}", bufs=1) as accpool:
                    gcol = _emit_gather_range(
                        nc, tc, pool, h_dram, gidx1, gw1, sidx1, cc1_in,
                        calls1, gcol, sch, tlo, thi, N1, accpool, f"a1p{pi}")
                    sch += -(-(thi - tlo) // WMAX)

            nc.gpsimd.collective_compute(
                "ReduceScatter", mybir.AluOpType.add, replica_groups=rg,
                ins=[cc1_in[:, :]], outs=[cc1_out[:, :]],
            )

            # ---- layer-1 output: h1T = relu(Ws1.T@xToT + Wn1.T@aggT + b1)
            with tc.tile_pool(name="out1", bufs=1) as opool:
                xTo = opool.tile([128, 2 * DPC1], BF, tag="xTo")
                nc.sync.dma_start(xTo[:, 0:DPC1], xTo_t[0:128, :])
                nc.sync.dma_start(xTo[:, DPC1 : 2 * DPC1], xTo_t[128:256, :])
                aggT = opool.tile([128, 2 * DPC1], BF, tag="aggT")
                for r0 in range(0, DPC1, 128):
                    nr = min(128, DPC1 - r0)
                    ag = pool.tile([128, D], FP, tag="aggrow", name=f"ag_{r0}")
                    nc.sync.dma_start(ag[:nr, :], cc1_out[r0 : r0 + nr, :])
                    for hf in range(2):
                        pt = psum.tile([128, 128], FP, tag="tpose",
                                       name=f"pt_{r0}_{hf}")
                        nc.tensor.transpose(
                            pt[:, :nr], ag[:nr, hf * 128 : (hf + 1) * 128],
                            ident[:nr, :nr])
                        nc.vector.tensor_copy(
                            aggT[:, hf * DPC1 + r0 : hf * DPC1 + r0 + nr],
                            pt[:, :nr])
                for hf in range(2):
                    for n0 in range(0, DPC1, 512):
                        nn = min(512, DPC1 - n0)
                        ps = psum.tile([128, 512], FP, tag="out1",
                                       name=f"po_{hf}_{n0}")
                        for kf in range(2):
                            nc.tensor.matmul(
                                ps[:, :nn],
                                W["Ws1"][:, kf * D + hf * 128 :
                                         kf * D + (hf + 1) * 128],
                                xTo[:, kf * DPC1 + n0 : kf * DPC1 + n0 + nn],
                                start=(kf == 0), stop=False)
                        for kf in range(2):
                            nc.tensor.matmul(
                                ps[:, :nn],
                                W["Wn1"][:, kf * D + hf * 128 :
                                         kf * D + (hf + 1) * 128],
                                aggT[:, kf * DPC1 + n0 : kf * DPC1 + n0 + nn],
                                start=False, stop=(kf == 1))
                        nc.scalar.activation(
                            h1T[:, hf * DPC1 + n0 : hf * DPC1 + n0 + nn],
                            ps[:, :nn], Relu, bias=b1c[:, hf : hf + 1])

            # ---- layer-2 projection: h2 = relu(h1 @ Wp2 + bp2), row-major
            for r0 in range(0, DPC1, 128):
                nr = min(128, DPC1 - r0)
                ps = psum.tile([128, D], FP, tag="proj", name=f"ps2_{r0}")
                for kf in range(2):
                    nc.tensor.matmul(
                        ps[:nr, :],
                        h1T[:, kf * DPC1 + r0 : kf * DPC1 + r0 + nr],
                        W["Wp2"][:, kf * D : (kf + 1) * D],
                        start=(kf == 0), stop=False)
                if with_bias:
                    nc.tensor.matmul(ps[:nr, :], ones[:, :nr],
                                     brow["bp2r"][:, :], start=False, stop=True)
                h2row = pool.tile([128, D], BF, tag="hrow", name=f"h2_{r0}")
                nc.scalar.activation(h2row[:nr, :], ps[:nr, :], Relu)
                nc.sync.dma_start(h2_dram[r0 : r0 + nr, :], h2row[:nr, :])

            # ---- layer-2 gather/sum rounds + scatter
            gcol = 0
            sch = 0
            for pi, (tlo, thi) in enumerate(PHASES2):
                with tc.tile_pool(name=f"acc2_{pi}", bufs=1) as accpool:
                    gcol = _emit_gather_range(
                        nc, tc, pool, h2_dram, gidx2, gw2, sidx2, cc2_in,
                        calls2, gcol, sch, tlo, thi, N2, accpool, f"a2p{pi}")
                    sch += -(-(thi - tlo) // WMAX)

            nc.gpsimd.collective_compute(
                "ReduceScatter", mybir.AluOpType.add, replica_groups=rg,
                ins=[cc2_in[:, :]], outs=[cc2_out[:, :]],
            )

            # ---- layer-2 output: out = relu(h1[:4000]@Ws2 + agg2@Wn2 + b2)
            agg2T = cpool.tile([128, 2 * DPC2], BF, tag="agg2T")
            for r0 in range(0, DPC2, 128):
                nr = min(128, DPC2 - r0)
                ag = pool.tile([128, D], FP, tag="aggrow", name=f"ag2_{r0}")
                nc.sync.dma_start(ag[:nr, :], cc2_out[r0 : r0 + nr, :])
                for hf in range(2):
                    pt = psum.tile([128, 128], FP, tag="tpose",
                                   name=f"pt2_{r0}_{hf}")
                    nc.tensor.transpose(
                        pt[:, :nr], ag[:nr, hf * 128 : (hf + 1) * 128],
                        ident[:nr, :nr])
                    nc.vector.tensor_copy(
                        agg2T[:, hf * DPC2 + r0 : hf * DPC2 + r0 + nr],
                        pt[:, :nr])
            for r0 in range(0, DPC2, 128):
                nr = min(128, DPC2 - r0)
                ps = psum.tile([128, D], FP, tag="proj", name=f"pso_{r0}")
                for kf in range(2):
                    nc.tensor.matmul(
                        ps[:nr, :],
                        h1T[:, kf * DPC1 + r0 : kf * DPC1 + r0 + nr],
                        W["Ws2"][:, kf * D : (kf + 1) * D],
                        start=(kf == 0), stop=False)
                for kf in range(2):
                    nc.tensor.matmul(
                        ps[:nr, :],
                        agg2T[:, kf * DPC2 + r0 : kf * DPC2 + r0 + nr],
                        W["Wn2"][:, kf * D : (kf + 1) * D],
                        start=False, stop=(kf == 1 and not with_bias))
                if with_bias:
                    nc.tensor.matmul(ps[:nr, :], ones[:, :nr], brow["b2r"][:, :],
                                     start=False, stop=True)
                orow = pool.tile([128, D], FP, tag="orow", name=f"o_{r0}")
                nc.scalar.activation(orow[:nr, :], ps[:nr, :], Relu)
                nc.sync.dma_start(out_t[r0 : r0 + nr, :], orow[:nr, :])

            for nm, t in [("h_dram", h_dram), ("cc1_in", cc1_in),
                          ("cc1_out", cc1_out), ("h2_dram", h2_dram),
                          ("cc2_in", cc2_in), ("cc2_out", cc2_out)]:
                if debug:
                    nc.sync.dma_start(dbg[nm][:, :], t[:, :])

    nc.compile()
    return nc


_CACHE = {}


def kernel(**inputs) -> np.ndarray:
    in_maps, calls1, calls2 = _prep(inputs)
    with_bias = any(
        np.any(np.asarray(inputs[b])) for b in ("bp1", "b1", "bp2", "b2"))
    key = (tuple(calls1), tuple(calls2), with_bias)
    if key not in _CACHE:
        _CACHE[key] = _build_program(calls1, calls2, with_bias=with_bias)
    nc = _CACHE[key]
    res = run_bass_kernel_spmd(nc, in_maps, core_ids=list(range(NC)))
    return np.concatenate([res.results[c]["out"] for c in range(NC)], axis=0)
